# revision 1
# baseline (speedup 1.0000x reference)
"""Two-layer GAT (PyG GATConv semantics) on 8 Trainium2 NeuronCores.

Strategy (dst-sharded, edge chunks of 128 on partitions):
- Shard destination nodes contiguously across the 8 cores (6250 each).
- Host preprocessing (integer graph data only): add self-loops, sort edges
  by dst, split per core, group per 128-dst tile, pad each tile's edge runs
  to multiples of 128, and build per-chunk selection matrices
  (Sel [128e x 128j] / SelT [128j x 128e]) plus src-index lists.
- Device, per layer: project node features (h = x @ W plus folded attention
  logit columns and skip projection), AllGather the per-node table rows
  [h(128) | a_s(2)], then per dst tile: indirect-DMA gather of src rows,
  per-edge attention weights w = max(exp(z), exp(0.2 z)) with z = a_s + a_d
  (exp(leaky_relu) factorization; softmax scale-invariance makes the max
  subtraction unnecessary), weighted aggregation + denominators via PE
  matmuls accumulating in PSUM, then normalize, add skip, relu.
"""

import sys

if "/opt/trn_rl_repo" not in sys.path:
    sys.path.insert(0, "/opt/trn_rl_repo")

import numpy as np

import concourse.bass as bass
import concourse.mybir as mybir
import concourse.tile as tile
from concourse.bass_utils import run_bass_kernel_spmd
from concourse.masks import make_identity

N, E, F_IN, H, C = 50000, 800000, 128, 2, 64
HC = H * C
NCORES = 8
SHARD = N // NCORES            # 6250
P = 128
TILES = (SHARD + P - 1) // P   # 49
NPAD = TILES * P               # 6272
ROW = 132                      # table row: h(128) | a_s(2) | pad(2)
PRJ = 260                      # proj cols: W(128) | w_as(2) | w_ad(2) | Wsk(128)

F32 = mybir.dt.float32
BF16 = mybir.dt.bfloat16
I32 = mybir.dt.int32
NP_BF16 = mybir.dt.np(BF16)


def _split_sync_waits(nc, limit=1):
    """walrus in this container rejects >1 sync wait per instruction; move
    excess waits onto NoOps inserted just before the offending one."""
    ctr = [0]

    def fresh_noop(engine, waits):
        ctr[0] += 1
        return mybir.InstNoOp(
            name=f"waitsplit-{ctr[0]}",
            engine=engine,
            bass_nofuse=True,
            sync_info=mybir.SyncInfo(on_wait=list(waits), on_update=[]),
        )

    for f in nc.m.functions:
        for bb in f.blocks:
            out = []
            changed = False
            for ins in bb.instructions:
                si = ins.sync_info
                waits = list(si.on_wait) if si else []
                if len(waits) > limit:
                    changed = True
                    excess, keep = waits[:-limit], waits[-limit:]
                    for i in range(0, len(excess), limit):
                        noop = fresh_noop(ins.engine, excess[i : i + limit])
                        nc.register_instruction(noop, overwrite=True)
                        out.append(noop)
                    ins.sync_info = mybir.SyncInfo(
                        on_wait=keep, on_update=list(si.on_update)
                    )
                out.append(ins)
            if changed:
                bb.instructions = out
    return ctr[0]


def _host_prep(src, dst):
    s = np.concatenate([src.astype(np.int64), np.arange(N, dtype=np.int64)])
    d = np.concatenate([dst.astype(np.int64), np.arange(N, dtype=np.int64)])
    order = np.argsort(d, kind="stable")
    s, d = s[order], d[order]

    # boundaries of every (core, tile) group in the dst-sorted edge list
    bounds = np.empty(NCORES * TILES + 1, np.int64)
    k = 0
    for c in range(NCORES):
        for t in range(TILES):
            lo = c * SHARD + min(t * P, SHARD)
            bounds[k] = np.searchsorted(d, lo, side="left")
            k += 1
    bounds[-1] = len(d)

    cnt = np.diff(bounds).reshape(NCORES, TILES)
    cpt = np.maximum((cnt + P - 1) // P, 1).max(axis=0)   # per-tile, cross-core
    choff = np.concatenate([[0], np.cumsum(cpt)]).astype(np.int64)
    totch = int(choff[-1])

    idx_all = np.zeros((NCORES, P, totch), np.int32)
    selt_all = np.zeros((NCORES, P, totch * P), np.float32)
    sel_all = np.zeros((NCORES, P, totch * P), NP_BF16)
    one = np.ones((), NP_BF16)
    for c in range(NCORES):
        for t in range(TILES):
            b0 = bounds[c * TILES + t]
            b1 = bounds[c * TILES + t + 1]
            n = b1 - b0
            if n == 0:
                continue
            es = s[b0:b1]
            jl = d[b0:b1] - (c * SHARD + t * P)
            off = choff[t]
            ch = np.arange(n) // P
            ep = np.arange(n) % P
            idx_all[c, ep, off + ch] = es
            selt_all[c, jl, (off + ch) * P + ep] = 1.0
            sel_all[c, ep, (off + ch) * P + jl] = one
    return cpt, choff, totch, idx_all, selt_all, sel_all


def _fold_weights(W, att_src, att_dst, Wsk):
    w_as = np.stack([W[:, h * C:(h + 1) * C] @ att_src[h] for h in range(H)], 1)
    w_ad = np.stack([W[:, h * C:(h + 1) * C] @ att_dst[h] for h in range(H)], 1)
    return np.concatenate([W, w_as, w_ad, Wsk], axis=1).astype(np.float32)


def _build_nc(cpt, choff, totch):
    nc = bass.Bass(
        "TRN2",
        num_devices=NCORES,
        use_seq_codegen=True,
        dynamic_dma_scratch_size=131072,
    )
    xs = nc.dram_tensor("xs", [NPAD, F_IN], F32, kind="ExternalInput")
    idx = nc.dram_tensor("idx", [P, totch], I32, kind="ExternalInput")
    selt = nc.dram_tensor("selt", [P, totch * P], F32, kind="ExternalInput")
    sel = nc.dram_tensor("sel", [P, totch * P], BF16, kind="ExternalInput")
    wall1 = nc.dram_tensor("wall1", [F_IN, PRJ], F32, kind="ExternalInput")
    wall2 = nc.dram_tensor("wall2", [HC, PRJ], F32, kind="ExternalInput")
    bb1 = nc.dram_tensor("bb1", [P, HC], F32, kind="ExternalInput")
    bb2 = nc.dram_tensor("bb2", [P, HC], F32, kind="ExternalInput")
    out = nc.dram_tensor("out", [SHARD, HC], F32, kind="ExternalOutput")

    layers = []
    for li in (1, 2):
        cc_in = nc.dram_tensor(f"cc_in{li}", [SHARD, ROW], F32, kind="Internal")
        table = nc.dram_tensor(
            f"table{li}", [N, ROW], F32, kind="Internal", addr_space="Shared"
        )
        ad = nc.dram_tensor(f"ad{li}", [NPAD, 2], F32, kind="Internal")
        skipb = nc.dram_tensor(f"skipb{li}", [NPAD, HC], F32, kind="Internal")
        layers.append((cc_in, table, ad, skipb))

    with tile.TileContext(nc) as tc:
        with (
            tc.tile_pool(name="const", bufs=1) as constp,
            tc.tile_pool(name="proj", bufs=6) as projp,
            tc.tile_pool(name="ppsum", bufs=2, space="PSUM") as ppsum,
            tc.tile_pool(name="gath", bufs=2) as gathp,
            tc.tile_pool(name="selp", bufs=2) as selp,
            tc.tile_pool(name="small", bufs=5) as smallp,
            tc.tile_pool(name="fwp", bufs=2) as fwp,
            tc.tile_pool(name="apsum", bufs=2, space="PSUM") as apsum,
            tc.tile_pool(name="finp", bufs=4) as finp,
        ):
            ident = constp.tile([P, P], F32)
            make_identity(nc, ident[:])
            walls = {}
            bbs = {}
            for li, wsrc, bsrc in ((1, wall1, bb1), (2, wall2, bb2)):
                wt = constp.tile([P, PRJ], F32, tag=f"wall{li}")
                nc.sync.dma_start(out=wt[:], in_=wsrc[:])
                bt = constp.tile([P, HC], F32, tag=f"bb{li}")
                nc.sync.dma_start(out=bt[:], in_=bsrc[:])
                walls[li] = wt
                bbs[li] = bt

            # zero pad tails of the a_d arrays (layer-2 writes only valid
            # rows; uninitialized DRAM could be NaN and would poison the
            # expansion matmul via 0*NaN)
            zt = constp.tile([P, HC], F32, tag="zero")
            nc.vector.memset(zt[:], 0.0)
            for li in (1, 2):
                nc.sync.dma_start(
                    out=layers[li - 1][2][SHARD:NPAD, :],
                    in_=zt[: NPAD - SHARD, :2],
                )

            # whole-layer gather index tile, loaded once
            it_all = constp.tile([P, totch], I32, tag="itall")
            nc.sync.dma_start(out=it_all[:], in_=idx[:])

            def proj_tile(li, t, xt):
                cc_in, table, ad, skipb = layers[li - 1]
                wt = walls[li]
                bt = bbs[li]
                if True:
                    rows = min(P, SHARD - t * P)
                    tp = ppsum.tile([P, F_IN], F32, tag="tp")
                    nc.tensor.transpose(out=tp[:], in_=xt[:], identity=ident[:])
                    xT = projp.tile([P, F_IN], F32, tag="xT")
                    nc.vector.tensor_copy(out=xT[:], in_=tp[:])
                    pj = ppsum.tile([P, PRJ], F32, tag="pj")
                    nc.tensor.matmul(
                        out=pj[:], lhsT=xT[:], rhs=wt[:], start=True, stop=True
                    )
                    rowst = projp.tile([P, ROW], F32, tag="rowst")
                    nc.vector.tensor_copy(out=rowst[:, 0:130], in_=pj[:, 0:130])
                    adt = projp.tile([P, 2], F32, tag="adt")
                    nc.scalar.copy(out=adt[:], in_=pj[:, 130:132])
                    skl = projp.tile([P, HC], F32, tag="skl")
                    nc.vector.tensor_add(
                        out=skl[:], in0=pj[:, 132:260], in1=bt[:]
                    )
                    nc.sync.dma_start(
                        out=cc_in[t * P : t * P + rows, :], in_=rowst[:rows, :]
                    )
                    nc.scalar.dma_start(
                        out=ad[t * P : t * P + rows, :], in_=adt[:rows, :]
                    )
                    nc.scalar.dma_start(
                        out=skipb[t * P : (t + 1) * P, :], in_=skl[:]
                    )

            def proj_collective(li):
                cc_in, table, ad, skipb = layers[li - 1]
                nc.gpsimd.collective_compute(
                    "AllGather",
                    mybir.AluOpType.bypass,
                    replica_groups=[list(range(NCORES))],
                    ins=[cc_in[:]],
                    outs=[table[:]],
                )

            def sweep(li, dst_dram, relu):
                cc_in, table, ad, skipb = layers[li - 1]
                for t in range(TILES):
                    rows = min(P, SHARD - t * P)
                    tt = int(cpt[t])
                    off = int(choff[t])
                    st = selp.tile([P, tt * P], F32, tag="selt")
                    nc.scalar.dma_start(
                        out=st[:], in_=selt[:, off * P : (off + tt) * P]
                    )
                    se = selp.tile([P, tt * P], BF16, tag="sel")
                    nc.sync.dma_start(
                        out=se[:], in_=sel[:, off * P : (off + tt) * P]
                    )
                    v2 = smallp.tile([P, 2], F32, tag="v2")
                    nc.scalar.dma_start(out=v2[:], in_=ad[t * P : (t + 1) * P, :])

                    gt = gathp.tile([P, tt, ROW], F32, tag="gt")
                    zp = apsum.tile([P, tt * 2], F32, tag="zp")
                    agg = apsum.tile([P, 130], F32, tag="agg")
                    for k in range(tt):
                        nc.gpsimd.indirect_dma_start(
                            out=gt[:, k, :],
                            out_offset=None,
                            in_=table[:],
                            in_offset=bass.IndirectOffsetOnAxis(
                                ap=it_all[:, off + k : off + k + 1], axis=0
                            ),
                        )
                        nc.tensor.matmul(
                            out=zp[:, 2 * k : 2 * k + 2],
                            lhsT=st[:, k * P : (k + 1) * P],
                            rhs=v2[:],
                            start=True,
                            stop=True,
                        )
                    z = smallp.tile([P, tt, 2], F32, tag="z")
                    nc.vector.tensor_add(
                        out=z[:],
                        in0=zp[:].rearrange("p (t two) -> p t two", two=2),
                        in1=gt[:, :, 128:130],
                    )
                    w1 = smallp.tile([P, tt, 2], F32, tag="w1")
                    nc.scalar.activation(
                        out=w1[:], in_=z[:], func=mybir.ActivationFunctionType.Exp
                    )
                    w2 = smallp.tile([P, tt, 2], F32, tag="w2")
                    nc.scalar.activation(
                        out=w2[:],
                        in_=z[:],
                        func=mybir.ActivationFunctionType.Exp,
                        scale=0.2,
                    )
                    w = smallp.tile([P, tt, 2], F32, tag="w")
                    nc.vector.tensor_tensor(
                        out=w[:], in0=w1[:], in1=w2[:], op=mybir.AluOpType.max
                    )
                    fw = fwp.tile([P, tt, 130], BF16, tag="fw")
                    nc.vector.tensor_copy(out=fw[:, :, 128:130], in_=w[:])
                    for k in range(tt):
                        for hh in range(H):
                            nc.vector.tensor_scalar_mul(
                                out=fw[:, k, hh * C : (hh + 1) * C],
                                in0=gt[:, k, hh * C : (hh + 1) * C],
                                scalar1=w[:, k, hh : hh + 1],
                            )
                        nc.tensor.matmul(
                            out=agg[:],
                            lhsT=se[:, k * P : (k + 1) * P],
                            rhs=fw[:, k, :],
                            start=(k == 0),
                            stop=(k == tt - 1),
                        )
                    rec = finp.tile([P, 2], F32, tag="rec")
                    nc.vector.reciprocal(out=rec[:], in_=agg[:, 128:130])
                    ot = finp.tile([P, HC], F32, tag="ot")
                    for hh in range(H):
                        nc.vector.tensor_scalar_mul(
                            out=ot[:, hh * C : (hh + 1) * C],
                            in0=agg[:, hh * C : (hh + 1) * C],
                            scalar1=rec[:, hh : hh + 1],
                        )
                    skl = finp.tile([P, HC], F32, tag="skl2")
                    nc.sync.dma_start(
                        out=skl[:], in_=skipb[t * P : (t + 1) * P, :]
                    )
                    ot2 = finp.tile([P, HC], F32, tag="ot2")
                    nc.vector.tensor_add(out=ot2[:], in0=ot[:], in1=skl[:])
                    if relu:
                        ot3 = finp.tile([P, HC], F32, tag="ot3")
                        nc.scalar.activation(
                            out=ot3[:],
                            in_=ot2[:],
                            func=mybir.ActivationFunctionType.Relu,
                        )
                    else:
                        ot3 = ot2
                    if dst_dram is not None:
                        nc.sync.dma_start(
                            out=dst_dram[t * P : t * P + rows, :],
                            in_=ot3[:rows, :],
                        )
                    else:
                        # layer-1: feed the tile straight into the layer-2
                        # projection (no HBM roundtrip)
                        proj_tile(2, t, ot3)

            def projection(li, src_dram):
                for t in range(TILES):
                    xt = projp.tile([P, F_IN], F32, tag="xt")
                    nc.scalar.dma_start(
                        out=xt[:], in_=src_dram[t * P : (t + 1) * P, :]
                    )
                    proj_tile(li, t, xt)

            projection(1, xs)
            proj_collective(1)
            sweep(1, None, relu=True)
            proj_collective(2)
            sweep(2, out, relu=False)

    _split_sync_waits(nc, limit=1)
    return nc


_CACHE = {}


def _get_program(src, dst):
    key = (hash(src.tobytes()), hash(dst.tobytes()))
    if key not in _CACHE:
        cpt, choff, totch, idx_all, selt_all, sel_all = _host_prep(src, dst)
        nc = _build_nc(cpt, choff, totch)
        _CACHE[key] = (nc, idx_all, selt_all, sel_all)
    return _CACHE[key]


def _run(inputs, trace=False):
    src = np.asarray(inputs["src"])
    dst = np.asarray(inputs["dst"])
    nc, idx_all, selt_all, sel_all = _get_program(src, dst)

    x = np.asarray(inputs["x"], np.float32)
    wall1 = _fold_weights(
        np.asarray(inputs["W1"]), np.asarray(inputs["att_src1"]),
        np.asarray(inputs["att_dst1"]), np.asarray(inputs["Wsk1"]),
    )
    wall2 = _fold_weights(
        np.asarray(inputs["W2"]), np.asarray(inputs["att_src2"]),
        np.asarray(inputs["att_dst2"]), np.asarray(inputs["Wsk2"]),
    )
    bb1 = np.tile(
        (np.asarray(inputs["b1"]) + np.asarray(inputs["bsk1"]))[None, :], (P, 1)
    ).astype(np.float32)
    bb2 = np.tile(
        (np.asarray(inputs["b2"]) + np.asarray(inputs["bsk2"]))[None, :], (P, 1)
    ).astype(np.float32)

    in_maps = []
    for c in range(NCORES):
        xsv = np.zeros((NPAD, F_IN), np.float32)
        xsv[:SHARD] = x[c * SHARD : (c + 1) * SHARD]
        in_maps.append(
            {
                "xs": xsv,
                "idx": idx_all[c],
                "selt": selt_all[c],
                "sel": sel_all[c],
                "wall1": wall1,
                "wall2": wall2,
                "bb1": bb1,
                "bb2": bb2,
            }
        )
    res = run_bass_kernel_spmd(
        nc, in_maps, core_ids=list(range(NCORES)), trace=trace
    )
    outp = np.concatenate([res.results[c]["out"] for c in range(NCORES)], axis=0)
    return outp.astype(np.float32), res.exec_time_ns


def kernel(**inputs) -> np.ndarray:
    out, _ = _run(inputs, trace=False)
    return out


def kernel_traced(**inputs):
    return _run(inputs, trace=True)

